# revision 13
# baseline (speedup 1.0000x reference)
"""CTDG encoder (exp-decay memory GNN) on 8 Trainium2 NeuronCores.

Strategy (pure node-parallel, per the natural sharding of this module):
- Host: shard the 200k nodes into 8 contiguous ranges of 25000 (padded to
  25088 = 49*512), route each event (unique_sources row) to its owning
  shard, and permute each shard so event nodes come first.  The event
  region is padded to a uniform multiple of 512 with identity events
  (msg=0, ts=last_update), so every 512-node device tile is either fully
  "event" or fully "plain".  memory/static_emb/messages are pre-transposed
  to feature-major [128, nodes] so the device never transposes.
- Device (SPMD, identical program, per-core data):
  Pass A: per-node scalars in tile-row layout [49, 512]:
      decay = exp((lu - ts)/30), rc = 1/(cnt_new + eps),
      ds = (1 - e_lamb) * exp((upd_lu - now)/30)   (as exp(x/30 + bias))
  Pass B: for each of 49 tiles of 512 nodes:
      per-node scales broadcast to [128, 512] via K=1 fp32r matmuls,
      event update + count-normalize on DVE, two-layer MLP on PE (fp32r),
      LeakyReLU (+bias) on ACT, final convex combine with static_emb on
      GPSIMD, all streamed against DMA.
- Host: inverse-permute and concatenate shard outputs.
"""

import numpy as np

import concourse.bacc as bacc
import concourse.tile as tile
from concourse import mybir
from concourse.bass_utils import run_bass_kernel_spmd

N_NODES = 200000
D = 128
NCORES = 8
S = N_NODES // NCORES          # 25000 real nodes per core
TILE = 512
NT = (S + TILE - 1) // TILE    # 49 tiles
S_PAD = NT * TILE              # 25088
LAMB = 30.0                    # memory-updater decay constant
OUTPUT = 30.0                  # embedding time-decay constant
EPS = 1e-10
SLOPE = 0.01

F32 = mybir.dt.float32
F32R = mybir.dt.float32r


def _build(NE, e_lamb, now_time):
    """Build the per-core bass program. NE = number of event tiles."""
    nc = bacc.Bacc("TRN2", target_bir_lowering=False, debug=False,
                   num_devices=NCORES)
    E_PAD = NE * TILE

    msumT_d = nc.dram_tensor("msumT", [D, S_PAD], F32R, kind="ExternalInput")
    # staticT is pre-scaled by e_lamb on the host (constant folding)
    staticT_d = nc.dram_tensor("staticT", [D, S_PAD], F32, kind="ExternalInput")
    msgT_d = nc.dram_tensor("msgT", [D, E_PAD], F32R, kind="ExternalInput")
    lu_d = nc.dram_tensor("lu_t", [NT, TILE], F32, kind="ExternalInput")
    ts_d = nc.dram_tensor("ts_t", [NE, TILE], F32, kind="ExternalInput")
    cnt_d = nc.dram_tensor("cnt_t", [NT, TILE], F32, kind="ExternalInput")
    msgc_d = nc.dram_tensor("msgc_t", [NE, TILE], F32, kind="ExternalInput")
    w1a_d = nc.dram_tensor("w1a", [D, D], F32R, kind="ExternalInput")
    w1b_d = nc.dram_tensor("w1b", [D, D], F32R, kind="ExternalInput")
    w2_d = nc.dram_tensor("w2", [D, D], F32R, kind="ExternalInput")
    b1_d = nc.dram_tensor("b1", [D, 1], F32, kind="ExternalInput")
    b2_d = nc.dram_tensor("b2", [D, 1], F32, kind="ExternalInput")
    ones_d = nc.dram_tensor("ones", [1, D], F32R, kind="ExternalInput")
    outT_d = nc.dram_tensor("outT", [D, S_PAD], F32, kind="ExternalOutput")

    # ds = exp(upd_lu/30 - now/30 + ln(1-e_lamb))
    one_m_el = max(1.0 - float(e_lamb), 1e-38)
    ds_bias = float(np.log(one_m_el) - float(now_time) / OUTPUT)
    inv_out = 1.0 / OUTPUT
    inv_lamb = 1.0 / LAMB

    with tile.TileContext(nc) as tc:
        with (
            tc.tile_pool(name="singles", bufs=1) as singles,
            tc.tile_pool(name="resid", bufs=1) as resid,
            tc.tile_pool(name="passa", bufs=1) as passa,
            tc.tile_pool(name="io", bufs=3) as io,
            tc.tile_pool(name="mid", bufs=3) as mid,
            tc.tile_pool(name="psb", bufs=4, space="PSUM") as psb,
            tc.tile_pool(name="psm", bufs=3, space="PSUM") as psm,
            tc.tile_pool(name="dram", bufs=1, space="DRAM") as dram,
        ):
            # ---- constants ----
            ones = singles.tile([1, D], F32R)
            w1a = singles.tile([D, D], F32R)
            w1b = singles.tile([D, D], F32R)
            w2 = singles.tile([D, D], F32R)
            b1 = singles.tile([D, 1], F32)
            b2 = singles.tile([D, 1], F32)
            nc.sync.dma_start(ones, ones_d[:, :])
            nc.sync.dma_start(w1a, w1a_d[:, :])
            nc.sync.dma_start(w1b, w1b_d[:, :])
            nc.sync.dma_start(w2, w2_d[:, :])
            nc.sync.dma_start(b1, b1_d[:, :])
            nc.sync.dma_start(b2, b2_d[:, :])

            # ---- pass A: per-node scalars, tile-row layout ----
            lu_t = passa.tile([NT, TILE], F32)
            ts_t = passa.tile([NE, TILE], F32)
            cnt_t = passa.tile([NT, TILE], F32)
            msgc_t = passa.tile([NE, TILE], F32)
            nc.sync.dma_start(lu_t, lu_d[:, :])
            nc.sync.dma_start(ts_t, ts_d[:, :])
            nc.sync.dma_start(cnt_t, cnt_d[:, :])
            nc.sync.dma_start(msgc_t, msgc_d[:, :])

            dec = resid.tile([NE, TILE], F32R)     # event decay
            rc = resid.tile([NT, TILE], F32R)      # 1/(cnt+eps)
            ds = resid.tile([NT, TILE], F32R)      # (1-e_lamb)*exp((ulu-now)/30)

            # (compute-engine instructions must start at partition 0/32/64,
            #  so: full-range [0:NT) op first, then event-range [0:NE)
            #  overwrite — both base partition 0)
            diff = passa.tile([NE, TILE], F32)
            nc.vector.tensor_sub(diff, lu_t[:NE, :], ts_t[:, :])
            nc.scalar.activation(dec, diff, mybir.ActivationFunctionType.Exp,
                                 scale=inv_lamb)
            # cnt_new = cnt*decay + msgc (event region), else cnt
            cn = passa.tile([NE, TILE], F32)
            nc.vector.tensor_mul(cn, cnt_t[:NE, :], dec.bitcast(F32))
            nc.vector.tensor_add(cn, cn, msgc_t[:, :])
            ce = passa.tile([NT, TILE], F32)
            nc.vector.tensor_scalar_add(ce, cnt_t, EPS)
            nc.vector.tensor_scalar_add(ce[:NE, :], cn, EPS)
            with nc.allow_low_precision(reason="fp32r rounding of 1/cnt is fine"):
                nc.vector.reciprocal(rc, ce)
            # ds: event rows use ts (= updated lu), plain rows use lu
            ds_bias_t = passa.tile([NT, 1], F32)
            nc.vector.memset(ds_bias_t, ds_bias)
            nc.scalar.activation(ds, lu_t,
                                 mybir.ActivationFunctionType.Exp,
                                 scale=inv_out, bias=ds_bias_t)
            nc.scalar.activation(ds[:NE, :], ts_t[:, :],
                                 mybir.ActivationFunctionType.Exp,
                                 scale=inv_out, bias=ds_bias_t[:NE, :])

            # Round-trip the per-node scalars through DRAM so pass B can
            # fetch each tile's three rows as one partition-0 [1, 3*TILE]
            # row (matmul operands must sit at base partition 0/32/64).
            scl = dram.tile([3, NT, TILE], F32R)
            nc.sync.dma_start(scl[0, :, :], rc)
            nc.sync.dma_start(scl[1, :, :], ds)
            nc.sync.dma_start(scl[2, :NE, :], dec)

            # ---- pass B: stream 49 tiles of 512 nodes ----
            for i in range(NT):
                ev = i < NE
                sl = slice(i * TILE, (i + 1) * TILE)

                ms = io.tile([D, TILE], F32R)
                nc.sync.dma_start(ms, msumT_d[:, sl])

                vrow = io.tile([1, 3 * TILE], F32R)
                if ev:
                    nc.sync.dma_start(vrow, scl[:, i, :])
                else:
                    nc.sync.dma_start(vrow[0:1, :2 * TILE], scl[:2, i, :])

                rc_b = psb.tile([D, TILE], F32, tag="bcast")
                nc.tensor.matmul(rc_b, ones, vrow[0:1, 0:TILE],
                                 start=True, stop=True)
                ds_b = psb.tile([D, TILE], F32, tag="bcast")
                nc.tensor.matmul(ds_b, ones, vrow[0:1, TILE:2 * TILE],
                                 start=True, stop=True)

                if ev:
                    dec_b = psb.tile([D, TILE], F32, tag="bcast")
                    nc.tensor.matmul(dec_b, ones, vrow[0:1, 2 * TILE:3 * TILE],
                                     start=True, stop=True)
                    # m2 = ms*decay + msg: multiply on DVE, then accumulate
                    # the message tile straight from DRAM (SWDGE compute-DMA)
                    m2 = mid.tile([D, TILE], F32R)
                    nc.vector.tensor_mul(m2, ms.bitcast(F32), dec_b)
                    nc.gpsimd.dma_start(m2, msgT_d[:, sl],
                                        accum_op=mybir.AluOpType.add)
                    ftop = mid.tile([D, TILE], F32R)
                    nc.vector.tensor_mul(ftop, m2.bitcast(F32), rc_b)
                    fbot = m2
                else:
                    ftop = mid.tile([D, TILE], F32R)
                    nc.vector.tensor_mul(ftop, ms.bitcast(F32), rc_b)
                    fbot = ms

                ps1 = psm.tile([D, TILE], F32, tag="mm")
                nc.tensor.matmul(ps1, w1a, ftop, start=True, stop=False)
                nc.tensor.matmul(ps1, w1b, fbot, start=False, stop=True)
                h1 = mid.tile([D, TILE], F32R)
                nc.scalar.activation(h1, ps1, mybir.ActivationFunctionType.Lrelu,
                                     bias=b1, scale=1.0, alpha=SLOPE)
                ps2 = psm.tile([D, TILE], F32, tag="mm")
                nc.tensor.matmul(ps2, w2, h1, start=True, stop=True)
                h2 = mid.tile([D, TILE], F32)
                nc.scalar.activation(h2, ps2, mybir.ActivationFunctionType.Lrelu,
                                     bias=b2, scale=1.0, alpha=SLOPE)
                # outt = h2*ds, then accumulate e_lamb*static from DRAM
                outt = mid.tile([D, TILE], F32)
                nc.vector.tensor_mul(outt, h2, ds_b)
                nc.gpsimd.dma_start(outt, staticT_d[:, sl],
                                    accum_op=mybir.AluOpType.add)
                nc.sync.dma_start(outT_d[:, sl], outt)

    nc.compile()
    return nc


def _preprocess(memory, last_update, unique_messages, unique_timestamps,
                static_emb, W1, b1, W2, b2, e_lamb, now_time, unique_sources):
    """Shard + route events + permute; returns (in_maps, perms, NE)."""
    memory = np.asarray(memory, dtype=np.float32)
    last_update = np.asarray(last_update, dtype=np.float32)
    unique_messages = np.asarray(unique_messages, dtype=np.float32)
    unique_timestamps = np.asarray(unique_timestamps, dtype=np.float32)
    static_emb = np.asarray(static_emb, dtype=np.float32)
    unique_sources = np.asarray(unique_sources)

    owner = unique_sources // S
    order = np.argsort(owner, kind="stable")
    counts = np.bincount(owner, minlength=NCORES)
    starts = np.concatenate([[0], np.cumsum(counts)])
    NE = int(np.ceil(max(1, counts.max()) / TILE))
    E_PAD = NE * TILE

    w1 = np.asarray(W1, dtype=np.float32)
    w1a = np.ascontiguousarray(w1[:D, :])
    w1b = np.ascontiguousarray(w1[D:, :])
    w2 = np.ascontiguousarray(np.asarray(W2, dtype=np.float32))
    b1c = np.asarray(b1, dtype=np.float32).reshape(D, 1).copy()
    b2c = np.asarray(b2, dtype=np.float32).reshape(D, 1).copy()
    ones = np.ones((1, D), dtype=np.float32)

    in_maps = []
    perms = []
    for c in range(NCORES):
        ev_rows = order[starts[c]:starts[c + 1]]
        src_local = unique_sources[ev_rows] - c * S
        E_c = src_local.shape[0]

        is_ev = np.zeros(S, dtype=bool)
        is_ev[src_local] = True
        non_ev = np.nonzero(~is_ev)[0]
        perm = np.concatenate([src_local, non_ev]).astype(np.int64)
        perms.append(perm)

        mem_pad = np.empty((S_PAD, D + 1), dtype=np.float32)
        mem_pad[:S] = memory[c * S:(c + 1) * S][perm]
        mem_pad[S:, :D] = 0.0
        mem_pad[S:, D] = 1.0
        lu_pad = np.zeros(S_PAD, dtype=np.float32)
        lu_pad[:S] = last_update[c * S:(c + 1) * S][perm]
        st_pad = np.zeros((S_PAD, D), dtype=np.float32)
        st_pad[:S] = static_emb[c * S:(c + 1) * S][perm]
        st_pad *= np.float32(e_lamb)   # fold e_lamb into the static table

        msg_full = np.zeros((E_PAD, D + 1), dtype=np.float32)
        msg_full[:E_c] = unique_messages[ev_rows]
        ts_full = np.empty(E_PAD, dtype=np.float32)
        ts_full[:E_c] = unique_timestamps[ev_rows]
        ts_full[E_c:] = lu_pad[E_c:E_PAD]   # identity events: ts = lu, msg = 0

        in_maps.append({
            "msumT": np.ascontiguousarray(mem_pad[:, :D].T),
            "staticT": np.ascontiguousarray(st_pad.T),
            "msgT": np.ascontiguousarray(msg_full[:, :D].T),
            "lu_t": lu_pad.reshape(NT, TILE).copy(),
            "ts_t": ts_full.reshape(NE, TILE).copy(),
            "cnt_t": mem_pad[:, D].reshape(NT, TILE).copy(),
            "msgc_t": msg_full[:, D].reshape(NE, TILE).copy(),
            "w1a": w1a, "w1b": w1b, "w2": w2,
            "b1": b1c, "b2": b2c, "ones": ones,
        })
    return in_maps, perms, NE


def _run(inputs, trace=False, trace_cores=None):
    in_maps, perms, NE = _preprocess(**inputs)
    nc = _build(NE, inputs["e_lamb"], inputs["now_time"])
    res = run_bass_kernel_spmd(nc, in_maps, core_ids=list(range(NCORES)),
                               trace=trace, trace_cores=trace_cores)
    out = np.empty((N_NODES, D), dtype=np.float32)
    for c in range(NCORES):
        out_perm = res.results[c]["outT"].T[:S]
        shard = np.empty((S, D), dtype=np.float32)
        shard[perms[c]] = out_perm
        out[c * S:(c + 1) * S] = shard
    return out, res


def kernel(**inputs) -> np.ndarray:
    out, _ = _run(inputs, trace=False)
    return out


# revision 15
# speedup vs baseline: 2.0732x; 2.0732x over previous
"""CTDG encoder (exp-decay memory GNN) on 8 Trainium2 NeuronCores.

Strategy (pure node-parallel, per the natural sharding of this module):
- Host: shard the 200k nodes into 8 contiguous ranges of 25000 (padded to
  25088 = 49*512), route each event (unique_sources row) to its owning
  shard, and permute each shard so event nodes come first.  The event
  region is padded to a uniform multiple of 512 with identity events
  (msg=0, ts=last_update), so every 512-node device tile is either fully
  "event" or fully "plain".  memory/static_emb/messages are pre-transposed
  to feature-major [128, nodes] (bf16) so the device never transposes.
- Device (SPMD, identical program, per-core data):
  Pass A: per-node scalars in tile-row layout [49, 512] (f32 math):
      decay = exp((lu - ts)/30), rc = 1/(cnt_new + eps),
      ds = (1 - e_lamb) * exp((upd_lu - now)/30)   (as exp(x/30 + bias))
    then round-tripped through DRAM (bf16) so pass B can fetch them as
    partition-0 rows.
  Pass B: for each of 49 tiles of 512 nodes:
      rc/ds broadcast to [128,512] SBUF via GPSIMD partition_broadcast,
      decay broadcast via K=1 bf16 matmul (PE), event update +
      count-normalize + output combine on DVE (bf16 2x), two-layer MLP on
      PE (bf16), LeakyReLU (+bias) on ACT, IO in 7-tile chunked DMAs.
- Host: inverse-permute, upcast, and concatenate shard outputs.
"""

import numpy as np
import ml_dtypes

import concourse.bacc as bacc
import concourse.tile as tile
from concourse import mybir
from concourse.bass_utils import run_bass_kernel_spmd

N_NODES = 200000
D = 128
NCORES = 8
S = N_NODES // NCORES          # 25000 real nodes per core
TILE = 512
NT = (S + TILE - 1) // TILE    # 49 tiles
S_PAD = NT * TILE              # 25088
CHT = 7                        # tiles per IO chunk
NCH = NT // CHT                # 7 chunks
CHW = CHT * TILE               # 3584 columns per chunk
LAMB = 30.0                    # memory-updater decay constant
OUTPUT = 30.0                  # embedding time-decay constant
EPS = 1e-10
SLOPE = 0.01

F32 = mybir.dt.float32
BF16 = mybir.dt.bfloat16
NP_BF16 = ml_dtypes.bfloat16


def _build(NE, e_lamb, now_time):
    """Build the per-core bass program. NE = number of event tiles."""
    nc = bacc.Bacc("TRN2", target_bir_lowering=False, debug=False,
                   num_devices=NCORES)
    E_PAD = NE * TILE

    msumT_d = nc.dram_tensor("msumT", [D, S_PAD], BF16, kind="ExternalInput")
    # staticT is pre-scaled by e_lamb on the host (constant folding)
    staticT_d = nc.dram_tensor("staticT", [D, S_PAD], BF16, kind="ExternalInput")
    msgT_d = nc.dram_tensor("msgT", [D, E_PAD], BF16, kind="ExternalInput")
    lu_d = nc.dram_tensor("lu_t", [NT, TILE], F32, kind="ExternalInput")
    ts_d = nc.dram_tensor("ts_t", [NE, TILE], F32, kind="ExternalInput")
    cnt_d = nc.dram_tensor("cnt_t", [NT, TILE], F32, kind="ExternalInput")
    msgc_d = nc.dram_tensor("msgc_t", [NE, TILE], F32, kind="ExternalInput")
    w1a_d = nc.dram_tensor("w1a", [D, D], BF16, kind="ExternalInput")
    w1b_d = nc.dram_tensor("w1b", [D, D], BF16, kind="ExternalInput")
    w2_d = nc.dram_tensor("w2", [D, D], BF16, kind="ExternalInput")
    b1_d = nc.dram_tensor("b1", [D, 1], F32, kind="ExternalInput")
    b2_d = nc.dram_tensor("b2", [D, 1], F32, kind="ExternalInput")
    ones_d = nc.dram_tensor("ones", [1, D], BF16, kind="ExternalInput")
    outT_d = nc.dram_tensor("outT", [D, S_PAD], BF16, kind="ExternalOutput")

    # ds = exp(upd_lu/30 - now/30 + ln(1-e_lamb))
    one_m_el = max(1.0 - float(e_lamb), 1e-38)
    ds_bias = float(np.log(one_m_el) - float(now_time) / OUTPUT)
    inv_out = 1.0 / OUTPUT
    inv_lamb = 1.0 / LAMB

    with tile.TileContext(nc) as tc:
        with (
            tc.tile_pool(name="singles", bufs=1) as singles,
            tc.tile_pool(name="passa", bufs=1) as passa,
            tc.tile_pool(name="io", bufs=2) as io,
            tc.tile_pool(name="vrows", bufs=2) as vrows,
            tc.tile_pool(name="mid", bufs=3) as mid,
            tc.tile_pool(name="bc", bufs=4) as bc,
            tc.tile_pool(name="psb", bufs=3, space="PSUM") as psb,
            tc.tile_pool(name="psm", bufs=4, space="PSUM") as psm,
            tc.tile_pool(name="dram", bufs=1, space="DRAM") as dram,
        ):
            # ---- constants ----
            ones = singles.tile([1, D], BF16)
            w1a = singles.tile([D, D], BF16)
            w1b = singles.tile([D, D], BF16)
            w2 = singles.tile([D, D], BF16)
            b1 = singles.tile([D, 1], F32)
            b2 = singles.tile([D, 1], F32)
            nc.sync.dma_start(ones, ones_d[:, :])
            nc.sync.dma_start(w1a, w1a_d[:, :])
            nc.sync.dma_start(w1b, w1b_d[:, :])
            nc.sync.dma_start(w2, w2_d[:, :])
            nc.sync.dma_start(b1, b1_d[:, :])
            nc.sync.dma_start(b2, b2_d[:, :])

            # ---- pass A: per-node scalars, tile-row layout ----
            lu_t = passa.tile([NT, TILE], F32)
            ts_t = passa.tile([NE, TILE], F32)
            cnt_t = passa.tile([NT, TILE], F32)
            msgc_t = passa.tile([NE, TILE], F32)
            nc.sync.dma_start(lu_t, lu_d[:, :])
            nc.sync.dma_start(ts_t, ts_d[:, :])
            nc.sync.dma_start(cnt_t, cnt_d[:, :])
            nc.sync.dma_start(msgc_t, msgc_d[:, :])

            dec = passa.tile([NE, TILE], BF16)     # event decay
            rc = passa.tile([NT, TILE], BF16)      # 1/(cnt+eps)
            ds = passa.tile([NT, TILE], BF16)      # (1-e_lamb)*exp((ulu-now)/30)

            # (compute-engine instructions must start at partition 0/32/64,
            #  so: full-range [0:NT) op first, then event-range [0:NE)
            #  overwrite — both base partition 0)
            diff = passa.tile([NE, TILE], F32)
            nc.vector.tensor_sub(diff, lu_t[:NE, :], ts_t[:, :])
            nc.scalar.activation(dec, diff, mybir.ActivationFunctionType.Exp,
                                 scale=inv_lamb)
            # cnt_new = cnt*decay + msgc (event region), else cnt
            cn = passa.tile([NE, TILE], F32)
            nc.vector.tensor_mul(cn, cnt_t[:NE, :], dec)
            nc.vector.tensor_add(cn, cn, msgc_t[:, :])
            ce = passa.tile([NT, TILE], F32)
            nc.vector.tensor_scalar_add(ce, cnt_t, EPS)
            nc.vector.tensor_scalar_add(ce[:NE, :], cn, EPS)
            with nc.allow_low_precision(reason="bf16 rounding of 1/cnt"):
                nc.vector.reciprocal(rc, ce)
            # ds: event rows use ts (= updated lu), plain rows use lu
            ds_bias_t = passa.tile([NT, 1], F32)
            nc.vector.memset(ds_bias_t, ds_bias)
            nc.scalar.activation(ds, lu_t,
                                 mybir.ActivationFunctionType.Exp,
                                 scale=inv_out, bias=ds_bias_t)
            nc.scalar.activation(ds[:NE, :], ts_t[:, :],
                                 mybir.ActivationFunctionType.Exp,
                                 scale=inv_out, bias=ds_bias_t[:NE, :])

            # Round-trip the per-node scalars through DRAM so pass B can
            # fetch them as partition-0 rows (PE/POOL broadcast sources).
            scl = dram.tile([3, NT, TILE], BF16)
            nc.sync.dma_start(scl[0, :, :], rc)
            nc.sync.dma_start(scl[1, :, :], ds)
            nc.sync.dma_start(scl[2, :NE, :], dec)
            # fill the unused tail of the dec plane (chunked reads touch it)
            if NE < NT:
                nc.sync.dma_start(scl[2, NE:, :], rc[NE:, :])

            # ---- pass B: 7 chunks of 7 tiles of 512 nodes ----
            for c in range(NCH):
                col0 = c * CHW
                csl = slice(col0, col0 + CHW)
                ms_ch = io.tile([D, CHW], BF16)
                nc.sync.dma_start(ms_ch, msumT_d[:, csl])
                st_ch = io.tile([D, CHW], BF16)
                nc.sync.dma_start(st_ch, staticT_d[:, csl])
                ev_tiles = max(0, min(NE - c * CHT, CHT))  # event tiles here
                if ev_tiles > 0:
                    mg_ch = io.tile([D, CHW], BF16)
                    nc.sync.dma_start(mg_ch[:, :ev_tiles * TILE],
                                      msgT_d[:, col0:col0 + ev_tiles * TILE])
                # scale rows for the chunk: layout [3 rows][CHT tiles][TILE]
                vch = vrows.tile([1, 3 * CHW], BF16)
                nc.sync.dma_start(vch, scl[:, c * CHT:(c + 1) * CHT, :])
                out_ch = io.tile([D, CHW], BF16)

                for j in range(CHT):
                    i = c * CHT + j
                    ev = i < NE
                    tsl = slice(j * TILE, (j + 1) * TILE)

                    def vrow(r):
                        off = (r * CHT + j) * TILE
                        return vch[0:1, off:off + TILE]

                    # rc/ds broadcasts on GPSIMD -> SBUF bf16
                    rc_bc = bc.tile([D, TILE], BF16, tag="rcbc")
                    nc.gpsimd.partition_broadcast(rc_bc, vrow(0))
                    ds_bc = bc.tile([D, TILE], BF16, tag="dsbc")
                    nc.gpsimd.partition_broadcast(ds_bc, vrow(1))

                    if ev:
                        # decay broadcast on PE -> PSUM f32
                        dec_b = psb.tile([D, TILE], F32, tag="decb")
                        nc.tensor.matmul(dec_b, ones, vrow(2),
                                         start=True, stop=True)
                        m2 = mid.tile([D, TILE], BF16)
                        nc.vector.tensor_mul(m2, ms_ch[:, tsl], dec_b)
                        m3 = mid.tile([D, TILE], BF16)
                        nc.vector.tensor_add(m3, m2, mg_ch[:, tsl])
                        ftop = mid.tile([D, TILE], BF16)
                        nc.vector.tensor_mul(ftop, m3, rc_bc)
                        fbot = m3
                    else:
                        ftop = mid.tile([D, TILE], BF16)
                        nc.vector.tensor_mul(ftop, ms_ch[:, tsl], rc_bc)
                        fbot = ms_ch[:, tsl]

                    ps1 = psm.tile([D, TILE], F32, tag="mm")
                    nc.tensor.matmul(ps1, w1a, ftop, start=True, stop=False)
                    nc.tensor.matmul(ps1, w1b, fbot, start=False, stop=True)
                    h1 = mid.tile([D, TILE], BF16)
                    nc.scalar.activation(h1, ps1,
                                         mybir.ActivationFunctionType.Lrelu,
                                         bias=b1, scale=1.0, alpha=SLOPE)
                    ps2 = psm.tile([D, TILE], F32, tag="mm")
                    nc.tensor.matmul(ps2, w2, h1, start=True, stop=True)
                    h2 = mid.tile([D, TILE], BF16)
                    nc.scalar.activation(h2, ps2,
                                         mybir.ActivationFunctionType.Lrelu,
                                         bias=b2, scale=1.0, alpha=SLOPE)
                    t2 = mid.tile([D, TILE], BF16)
                    nc.vector.tensor_mul(t2, h2, ds_bc)
                    nc.vector.tensor_add(out_ch[:, tsl], t2, st_ch[:, tsl])

                nc.sync.dma_start(outT_d[:, csl], out_ch)

    nc.compile()
    return nc


def _preprocess(memory, last_update, unique_messages, unique_timestamps,
                static_emb, W1, b1, W2, b2, e_lamb, now_time, unique_sources):
    """Shard + route events + permute; returns (in_maps, perms, NE)."""
    memory = np.asarray(memory, dtype=np.float32)
    last_update = np.asarray(last_update, dtype=np.float32)
    unique_messages = np.asarray(unique_messages, dtype=np.float32)
    unique_timestamps = np.asarray(unique_timestamps, dtype=np.float32)
    static_emb = np.asarray(static_emb, dtype=np.float32)
    unique_sources = np.asarray(unique_sources)

    owner = unique_sources // S
    order = np.argsort(owner, kind="stable")
    counts = np.bincount(owner, minlength=NCORES)
    starts = np.concatenate([[0], np.cumsum(counts)])
    NE = int(np.ceil(max(1, counts.max()) / TILE))
    E_PAD = NE * TILE

    w1 = np.asarray(W1, dtype=np.float32)
    w1a = np.ascontiguousarray(w1[:D, :]).astype(NP_BF16)
    w1b = np.ascontiguousarray(w1[D:, :]).astype(NP_BF16)
    w2 = np.ascontiguousarray(np.asarray(W2, dtype=np.float32)).astype(NP_BF16)
    b1c = np.asarray(b1, dtype=np.float32).reshape(D, 1).copy()
    b2c = np.asarray(b2, dtype=np.float32).reshape(D, 1).copy()
    ones = np.ones((1, D), dtype=NP_BF16)

    in_maps = []
    perms = []
    for c in range(NCORES):
        ev_rows = order[starts[c]:starts[c + 1]]
        src_local = unique_sources[ev_rows] - c * S
        E_c = src_local.shape[0]

        is_ev = np.zeros(S, dtype=bool)
        is_ev[src_local] = True
        non_ev = np.nonzero(~is_ev)[0]
        perm = np.concatenate([src_local, non_ev]).astype(np.int64)
        perms.append(perm)

        mem_pad = np.empty((S_PAD, D + 1), dtype=np.float32)
        mem_pad[:S] = memory[c * S:(c + 1) * S][perm]
        mem_pad[S:, :D] = 0.0
        mem_pad[S:, D] = 1.0
        lu_pad = np.zeros(S_PAD, dtype=np.float32)
        lu_pad[:S] = last_update[c * S:(c + 1) * S][perm]
        st_pad = np.zeros((S_PAD, D), dtype=np.float32)
        st_pad[:S] = static_emb[c * S:(c + 1) * S][perm]
        st_pad *= np.float32(e_lamb)   # fold e_lamb into the static table

        msg_full = np.zeros((E_PAD, D + 1), dtype=np.float32)
        msg_full[:E_c] = unique_messages[ev_rows]
        ts_full = np.empty(E_PAD, dtype=np.float32)
        ts_full[:E_c] = unique_timestamps[ev_rows]
        ts_full[E_c:] = lu_pad[E_c:E_PAD]   # identity events: ts = lu, msg = 0

        in_maps.append({
            "msumT": np.ascontiguousarray(mem_pad[:, :D].T).astype(NP_BF16),
            "staticT": np.ascontiguousarray(st_pad.T).astype(NP_BF16),
            "msgT": np.ascontiguousarray(msg_full[:, :D].T).astype(NP_BF16),
            "lu_t": lu_pad.reshape(NT, TILE).copy(),
            "ts_t": ts_full.reshape(NE, TILE).copy(),
            "cnt_t": mem_pad[:, D].reshape(NT, TILE).copy(),
            "msgc_t": msg_full[:, D].reshape(NE, TILE).copy(),
            "w1a": w1a, "w1b": w1b, "w2": w2,
            "b1": b1c, "b2": b2c, "ones": ones,
        })
    return in_maps, perms, NE


def _run(inputs, trace=False, trace_cores=None):
    in_maps, perms, NE = _preprocess(**inputs)
    nc = _build(NE, inputs["e_lamb"], inputs["now_time"])
    res = run_bass_kernel_spmd(nc, in_maps, core_ids=list(range(NCORES)),
                               trace=trace, trace_cores=trace_cores)
    out = np.empty((N_NODES, D), dtype=np.float32)
    for c in range(NCORES):
        out_perm = res.results[c]["outT"].T[:S].astype(np.float32)
        shard = np.empty((S, D), dtype=np.float32)
        shard[perms[c]] = out_perm
        out[c * S:(c + 1) * S] = shard
    return out, res


def kernel(**inputs) -> np.ndarray:
    out, _ = _run(inputs, trace=False)
    return out
